# revision 36
# baseline (speedup 1.0000x reference)
"""nn_Attn_9715216024104 — sparse attention (MLA + top-k select + sliding window).

Sharding: 2 cores = 2 batches; each core runs all 16 heads as 4 sequential
head-quads of the same Bass/Tile per-quad program (S^T layout, exp softmax
without max-subtraction — scores are <0.5 — with ones-column-folded Z rows
in the PV matmul). Fewer cores = fewer tunnel copies of the replicated
tensors (x, sel, rope tables) while head-sharded weights stay constant-size.

The axon tunnel runs at ~45 MB/s, so per-call host<->device bytes dominate
wall clock; this kernel minimizes them:
- Inputs are packed into 3 DRAM params (blob[128,NB], b96, b32) instead of 20.
- cq/ckv RMS-norm and the shared roped kr are computed ON DEVICE from x
  (norm weights folded into downstream projections on the host; rms factor
  via ones-matmul column reduce + Sqrt activation + DVE reciprocal + rank-1
  f32 matmul partition-broadcast).
- Rope cos/sin tables ship compact as [32,T] and are replicated/sign-folded
  on device; "swapped" rope projection weights are built on device by
  column-half swaps instead of being shipped twice.
- The epilogue (divide by Z, gate, sum the 3 branches, Wproj) runs ON
  DEVICE: branch gates are folded into the V projection weights on the
  host, evicts normalize by the PSUM Z row (DVE reciprocal + rank-1
  broadcast) and accumulate into a f32 [128,T] tile per quad; since a core
  holds all 16 heads, the final Wproj contraction also runs on device and
  the output is the finished bf16 [T, C] (shipped as [NKB,128,C]). Host
  epilogue is a pure reshape.
- kernel.py also enables JAX's persistent compilation cache and memoizes
  the BIR serialization so repeated run_bass_kernel_spmd calls skip the
  per-call XLA/NEFF rebuild.

Device layout notes:
- All matmul operands bf16 (rank-1 Z/rms broadcasts use f32); PSUM f32.
- Attention uses S^T tiles [k=128, q] so P^T feeds the PV matmul directly;
  V tiles carry a ones column so the PV matmul also produces Z rows.
- Rope is applied via duplicated "swapped" projection weights:
  rope(x) = x * cos + swap(x) * sgn*sin, with swap folded into a second
  matmul, so DVE only does 2 muls + 1 add.
- Causal / sliding-window masking is done on GPSIMD (affine_select zeroing
  of P^T after exp), keeping TensorE/ACT free of mask work.
"""

import math

import numpy as np
import ml_dtypes

try:  # persistent XLA compilation cache: the per-call jit is a fresh closure
    import jax

    jax.config.update("jax_compilation_cache_dir", "/tmp/jaxcache")
    jax.config.update("jax_persistent_cache_min_compile_time_secs", 0.0)
    jax.config.update("jax_persistent_cache_min_entry_size_bytes", 0)
except Exception:
    pass

BF = ml_dtypes.bfloat16

N_HEAD = 16
NOPE = 32
ROPE = 64
VDIM = 32
HD = NOPE + ROPE  # 96
WINDOW = 128
KEEP = 512
EPS = 1e-6
N_CORES = 2
HPC = 4  # heads per quad (the inner program unit)
QUADS = 4  # head-quads per core -> 16 heads per core
B, T, C = 2, 2048, 256
QT = 512  # q tile (free dim)
NJQ = T // QT  # 4 q tiles
NKB = T // 128  # 16 k blocks
SCALE = 1.0 / math.sqrt(HD)

# blob column offsets (all bf16, [128, NB]); per-quad weight sections are
# QW columns apart
XT = 0  # x^T, 2 cin chunks       [128, 2*T]
SEL = XT + 2 * T  # sel^T, 2 cin chunks     [128, 2*KEEP]
WSKN = SEL + 2 * KEEP  # Wsel_k nope, 2 chunks   [128, 2*128]
WSKR = WSKN + 256  # Wsel_k rope, 2 chunks   [128, 2*256]
WSV = WSKR + 512  # Wsel_v (gated), 2 chunks [128, 2*128]
WWKN = WSV + 256  # Wwin_k nope             [128, 2*128]
WWKR = WWKN + 256  # Wwin_k rope             [128, 2*256]
WWV = WWKR + 512  # Wwin_v (gated)          [128, 2*128]
QW = 2048  # per-quad stride of the WSKN..WWV block
WCQ = WSKN + QUADS * QW  # Wcq              [128, 2*96]
WCKV = WCQ + 192  # Wckv                    [128, 2*32]
WKR = WCKV + 64  # Wk_rope/N_HEAD          [128, 2*64]
WP = WKR + 128  # Wproj, 4 contraction chunks [128, 4*256]
NB = WP + 1024

# b96 columns ([96, N96]); per-quad stride QW96
WQN = 0  # Wq_nope (norm-folded)  [96, 128]
WQR = 128  # Wq_rope (norm-folded)  [96, 256]
QW96 = 384
N96 = QUADS * QW96

# b32 columns ([32, N32]); per-quad stride QW32 for the weight tail
C32 = 0  # cos^T [32, T]
S32 = T  # sin^T [32, T]
WKN = 2 * T  # Wk_nope (norm-folded) [32, 128]
WV = 2 * T + 128  # Wv (norm- and gate-folded) [32, 128]
QW32 = 256
N32 = 2 * T + QUADS * QW32

_CACHE = {}


# ---------------------------------------------------------------------------
# host-side helpers
# ---------------------------------------------------------------------------

def _freqs(t):
    f = 1.0 / 1e4 ** (np.arange(0, ROPE, 2, dtype=np.float32) / ROPE)
    ang = np.outer(np.arange(t, dtype=np.float32), f)
    return np.cos(ang).astype(np.float32), np.sin(ang).astype(np.float32)


# ---------------------------------------------------------------------------
# bass program (built once; identical for both cores)
# ---------------------------------------------------------------------------

def _build_bass(legalize=True):
    import concourse.bass as bass
    import concourse.mybir as mybir
    import concourse.tile as tile

    f32 = mybir.dt.float32
    bf16 = mybir.dt.bfloat16
    EXP = mybir.ActivationFunctionType.Exp
    SQRT = mybir.ActivationFunctionType.Sqrt
    GE = mybir.AluOpType.is_ge

    nc = bass.Bass(target_bir_lowering=False, debug=False)

    d_blob = nc.declare_dram_parameter("blob", [128, NB], bf16, isOutput=False)
    d_b96 = nc.declare_dram_parameter("b96", [96, N96], bf16, isOutput=False)
    d_b32 = nc.declare_dram_parameter("b32", [32, N32], bf16, isOutput=False)
    d_out = nc.declare_dram_parameter("outT", [NKB, 128, C], bf16,
                                      isOutput=True)

    def asl(base, s):  # absolute blob column slice
        return slice(base + s.start, base + s.stop)

    with tile.TileContext(nc) as tc:
        with (
            tc.tile_pool(name="const", bufs=1) as cpool,
            tc.tile_pool(name="big", bufs=1) as bpool,
            tc.tile_pool(name="pt", bufs=3) as ptpool,
            tc.tile_pool(name="sc", bufs=4) as scpool,
        ):
            _dma_engines = [nc.sync, nc.gpsimd, nc.scalar]
            _dma_rr = [0]

            def _dma(out, in_):
                eng = _dma_engines[_dma_rr[0] % len(_dma_engines)]
                _dma_rr[0] += 1
                eng.dma_start(out=out, in_=in_)

            s_blob = cpool.tile([128, NB], bf16, name="blob", tag="blob")
            s_b96 = cpool.tile([96, N96], bf16, name="b96", tag="b96")
            s_b32 = cpool.tile([32, N32], bf16, name="b32", tag="b32")

            # small/early-needed first; big x/sel tensors split for overlap
            _dma(s_b32[:, :], d_b32[:, :])
            _dma(s_b96[:, :], d_b96[:, :])
            _dma(s_blob[:, WSKN:NB], d_blob[:, WSKN:NB])
            _dma(s_blob[:, XT : XT + T], d_blob[:, XT : XT + T])
            _dma(s_blob[:, XT + T : XT + 2 * T], d_blob[:, XT + T : XT + 2 * T])
            _dma(s_blob[:, SEL : SEL + 2 * KEEP], d_blob[:, SEL : SEL + 2 * KEEP])

            def xT_ap(cc, ts_):
                return s_blob[:, asl(XT + cc * T, ts_)]

            # ---- rope tables [128, T] from compact [32, T] + sign folding ----
            c128 = cpool.tile([128, T], bf16, name="c128", tag="c128")
            s128 = cpool.tile([128, T], bf16, name="s128", tag="s128")
            for blk in range(4):
                rs = slice(32 * blk, 32 * blk + 32)
                nc.scalar.copy(c128[rs, :], s_b32[:, C32 : C32 + T])
                if blk % 2 == 0:
                    nc.vector.tensor_scalar_mul(
                        s128[rs, :], s_b32[:, S32 : S32 + T], -1.0)
                else:
                    nc.gpsimd.tensor_copy(s128[rs, :], s_b32[:, S32 : S32 + T])

            # ---- swapped rope weights built on device (column-half swap) ----
            wqrS = [cpool.tile([96, 256], bf16, name=f"wqrS{q}",
                               tag=f"wqrS{q}") for q in range(QUADS)]
            wskrS = [cpool.tile([128, 2, 256], bf16, name=f"wskrS{q}",
                                tag=f"wskrS{q}") for q in range(QUADS)]
            wwkrS = [cpool.tile([128, 2, 256], bf16, name=f"wwkrS{q}",
                                tag=f"wwkrS{q}") for q in range(QUADS)]
            wkrS = cpool.tile([128, 2, 64], bf16, tag="wkrS")
            _sw_rr = [0]

            def _sweng():
                eng = (nc.vector, nc.gpsimd, nc.scalar)[_sw_rr[0] % 3]
                _sw_rr[0] += 1
                return eng

            def swap_into(dst_ap, src_ap, nh):
                # both viewed [P, nh, 2, 32]; swap axis-2 halves
                dv = dst_ap.rearrange("p (h two c) -> p h two c", two=2, c=32)
                sv = src_ap.rearrange("p (h two c) -> p h two c", two=2, c=32)
                for half in range(2):
                    eng = _sweng()
                    (eng.tensor_copy if eng is not nc.scalar else eng.copy)(
                        dv[:, :, half, :], sv[:, :, 1 - half, :])

            for hq in range(QUADS):
                swap_into(wqrS[hq][:, :],
                          s_b96[:, hq * QW96 + WQR : hq * QW96 + WQR + 256], 4)
                for cc in range(2):
                    swap_into(
                        wskrS[hq][:, cc, :],
                        s_blob[:, hq * QW + WSKR + cc * 256 :
                               hq * QW + WSKR + cc * 256 + 256], 4)
                    swap_into(
                        wwkrS[hq][:, cc, :],
                        s_blob[:, hq * QW + WWKR + cc * 256 :
                               hq * QW + WWKR + cc * 256 + 256], 4)
            for cc in range(2):
                swap_into(wkrS[:, cc, :],
                          s_blob[:, WKR + cc * 64 : WKR + cc * 64 + 64], 1)

            # ---- constants for reductions/broadcasts ----
            ones96c = cpool.tile([96, 1], bf16, tag="o96c")
            ones32c = cpool.tile([32, 1], bf16, tag="o32c")
            ones96r = cpool.tile([1, 96], f32, tag="o96r")
            ones32r = cpool.tile([1, 32], f32, tag="o32r")
            epsc = cpool.tile([1, 1], f32, tag="epsc")
            nc.vector.memset(ones96c[:, :], 1.0)
            nc.vector.memset(ones32c[:, :], 1.0)
            nc.vector.memset(ones96r[:, :], 1.0)
            nc.vector.memset(ones32r[:, :], 1.0)
            nc.vector.memset(epsc[:, :], EPS)

            # ---- assembled per-head [96, h, T] q/k layouts ----
            cqT = bpool.tile([96, T], bf16)   # rms-normalized cq^T
            ckvT = bpool.tile([32, T], bf16)  # rms-normalized ckv^T
            q96 = bpool.tile([96, 4, T], bf16)
            k96 = bpool.tile([96, 4, T], bf16)     # branch 1 (kn | shared kr)
            ks96 = bpool.tile([96, 4, KEEP], bf16)  # branch 2
            kw96 = bpool.tile([96, 4, T], bf16)    # branch 3
            v1 = bpool.tile([128, NKB, 132], bf16)
            vs = bpool.tile([128, 4, 132], bf16)
            vw = bpool.tile([128, NKB, 132], bf16)
            acc = bpool.tile([128, T], f32)        # gated, normalized output
            # per-quad o^T (bf16) kept as Wproj matmul operands
            oq = [bpool.tile([128, T], bf16, name=f"oq{q}")
                  for q in range(QUADS)]

            with (
                tc.tile_pool(name="pp", bufs=2, space=bass.MemorySpace.PSUM) as pp,
                tc.tile_pool(name="sgp", bufs=2, space=bass.MemorySpace.PSUM) as sgp,
                tc.tile_pool(name="otp", bufs=2, space=bass.MemorySpace.PSUM) as otp,
            ):
                def rmsnorm_proj(dst, p, wcol, wwid, ones_col, ones_row, inv_n):
                    """dst[p, T] <- rms-normalized W^T x^T (tokens on free dim).
                    rms factor: ones-matmul column sum of squares -> Sqrt ACT
                    -> DVE reciprocal -> rank-1 f32 matmul broadcast."""
                    for t4 in range(NJQ):
                        ts_ = slice(t4 * QT, t4 * QT + QT)
                        ps = pp.tile([128, QT], f32, tag="p1",
                                     padded_shape=[128, QT])
                        for cc in range(2):
                            nc.tensor.matmul(
                                ps[0:p, :],
                                s_blob[:, wcol + cc * wwid : wcol + (cc + 1) * wwid],
                                xT_ap(cc, ts_), start=(cc == 0), stop=(cc == 1),
                            )
                        pre = scpool.tile([p, QT], bf16, tag="pre")
                        nc.scalar.copy(pre[:, :], ps[0:p, :])
                        sq = scpool.tile([p, QT], bf16, tag="sq")
                        nc.vector.tensor_mul(sq[:, :], pre[:, :], pre[:, :])
                        ps2 = pp.tile([1, QT], f32, tag="p1",
                                      padded_shape=[128, QT])
                        nc.tensor.matmul(ps2[:, :], ones_col[:, :], sq[:, :],
                                         start=True, stop=True)
                        srow = scpool.tile([1, QT], f32, tag="srow", bufs=2)
                        nc.scalar.activation(srow[:, :], ps2[:, :], SQRT,
                                             bias=epsc[:, :], scale=inv_n)
                        rin = scpool.tile([1, QT], f32, tag="rin", bufs=2)
                        nc.vector.reciprocal(rin[:, :], srow[:, :])
                        bc = pp.tile([128, QT], f32, tag="p1",
                                     padded_shape=[128, QT])
                        nc.tensor.matmul(bc[0:p, :], ones_row[:, 0:p],
                                         rin[:, :], start=True, stop=True)
                        nc.vector.tensor_mul(dst[:, ts_], pre[:, :], bc[0:p, :])

                def kr_build():
                    """k96[0:64, h, :] <- rope((x @ Wk_rope)/N_HEAD), all heads."""
                    for t4 in range(NJQ):
                        ts_ = slice(t4 * QT, t4 * QT + QT)
                        pr = pp.tile([128, QT], f32, tag="p1",
                                     padded_shape=[128, QT])
                        psw = pp.tile([128, QT], f32, tag="p1",
                                      padded_shape=[128, QT])
                        for cc in range(2):
                            nc.tensor.matmul(
                                pr[0:64, :],
                                s_blob[:, WKR + cc * 64 : WKR + cc * 64 + 64],
                                xT_ap(cc, ts_), start=(cc == 0), stop=(cc == 1),
                            )
                        for cc in range(2):
                            nc.tensor.matmul(
                                psw[0:64, :], wkrS[:, cc, :], xT_ap(cc, ts_),
                                start=(cc == 0), stop=(cc == 1),
                            )
                        t1 = scpool.tile([64, QT], bf16, tag="rt1")
                        t2 = scpool.tile([64, QT], bf16, tag="rt2")
                        nc.vector.tensor_mul(t1[:, :], pr[0:64, :],
                                             c128[0:64, ts_])
                        nc.vector.tensor_mul(t2[:, :], psw[0:64, :],
                                             s128[0:64, ts_])
                        nc.gpsimd.tensor_add(k96[0:64, 0, ts_], t1[:, :],
                                             t2[:, :])
                        for h in range(1, 4):
                            nc.scalar.copy(k96[0:64, h, ts_], k96[0:64, 0, ts_])

                def proj_nope(dest96, lhsW, rhs_of, tlen, nacc, eng=None):
                    """4-head nope projection, split per head into
                    dest96[64:96, h, ts]."""
                    step = min(tlen, QT)
                    for t4 in range(max(1, tlen // step)):
                        ts_ = slice(t4 * step, t4 * step + step)
                        ps = pp.tile([128, step], f32, tag="p1",
                                     padded_shape=[128, QT])
                        for cc in range(nacc):
                            nc.tensor.matmul(
                                ps[:], lhsW(cc), rhs_of(cc, ts_),
                                start=(cc == 0), stop=(cc == nacc - 1),
                            )
                        for h in range(4):
                            if eng == "act":
                                nc.scalar.copy(
                                    dest96[64:96, h, ts_],
                                    ps[32 * h : 32 * h + 32, :],
                                )
                            else:
                                nc.vector.tensor_copy(
                                    dest96[64:96, h, ts_],
                                    ps[32 * h : 32 * h + 32, :],
                                )

                def rope_proj(dest96, hpair, cos_sl, lhs_raw, lhs_sw,
                              rhs_list, tlen, ts_):
                    """Rope for one head-pair chunk; writes per-head rows
                    dest96[0:64, h, ts]."""
                    pr = pp.tile([128, tlen], f32, tag="p1",
                                 padded_shape=[128, QT])
                    psw = pp.tile([128, tlen], f32, tag="p1",
                                  padded_shape=[128, QT])
                    ncc = len(rhs_list)
                    for cc, rhs in enumerate(rhs_list):
                        nc.tensor.matmul(
                            pr[:], lhs_raw[cc], rhs,
                            start=(cc == 0), stop=(cc == ncc - 1),
                        )
                    for cc, rhs in enumerate(rhs_list):
                        nc.tensor.matmul(
                            psw[:], lhs_sw[cc], rhs,
                            start=(cc == 0), stop=(cc == ncc - 1),
                        )
                    t1 = scpool.tile([128, tlen], bf16, tag="rt1")
                    t2 = scpool.tile([128, tlen], bf16, tag="rt2")
                    nc.vector.tensor_mul(t1[:], pr[:], c128[:, cos_sl])
                    nc.vector.tensor_mul(t2[:], psw[:], s128[:, cos_sl])
                    for hi in range(2):
                        h = 2 * hpair + hi
                        hr = slice(64 * hi, 64 * hi + 64)
                        nc.gpsimd.tensor_add(
                            dest96[0:64, h, ts_], t1[hr, :], t2[hr, :]
                        )

                def rope_all(dest96, lhsW, lhsWS, rhs_of, tlen, nacc):
                    for j in range(2):
                        hs = slice(j * 128, j * 128 + 128)
                        step = min(tlen, QT)
                        for t4 in range(max(1, tlen // step)):
                            ts_ = slice(t4 * step, t4 * step + step)
                            rope_proj(
                                dest96, j, ts_,
                                [lhsW(cc, hs) for cc in range(nacc)],
                                [lhsWS(cc, hs) for cc in range(nacc)],
                                [rhs_of(cc, ts_) for cc in range(nacc)],
                                step, ts_,
                            )

                def v_tile(dest, nblk, lhs_fn, rhs_fn, nacc):
                    nc.vector.memset(dest[:, :, slice(32, 132, 33)], 1.0)
                    for tb in range(nblk):
                        ps = pp.tile([128, 128], f32, tag="p1",
                                     padded_shape=[128, QT])
                        for cc in range(nacc):
                            nc.tensor.matmul(
                                ps[:], lhs_fn(cc, tb), rhs_fn(cc),
                                start=(cc == 0), stop=(cc == nacc - 1),
                            )
                        nc.vector.tensor_copy(
                            dest[:, tb, :].rearrange(
                                "p (h c) -> p h c", h=4)[:, :, 0:32],
                            ps[:].rearrange("p (h c) -> p h c", h=4),
                        )

                def evict(jq, br, hp, ots):
                    """Normalize by the PSUM Z row and accumulate (gates are
                    folded into the V weights host-side)."""
                    oc = slice(jq * QT, jq * QT + QT)
                    for hi in range(2):
                        h = 2 * hp + hi
                        zi = scpool.tile([1, QT], f32, tag="zi", bufs=2)
                        nc.vector.reciprocal(zi[:, :], ots[hi][32:33, :])
                        zb = pp.tile([32, QT], f32, tag="p1",
                                     padded_shape=[128, QT])
                        nc.tensor.matmul(zb[:, :], ones32r[:, :], zi[:, :],
                                         start=True, stop=True)
                        # two PSUM srcs in one DVE op are illegal; stage zb
                        zbs = scpool.tile([32, QT], f32, tag="zbs", bufs=2)
                        nc.vector.tensor_copy(zbs[:, :], zb[:, :])
                        hr = slice(32 * h, 32 * h + 32)
                        ar = acc[hr, oc]
                        if br == 2:  # first writer of this acc region
                            nc.vector.tensor_mul(ar, ots[hi][0:32, :],
                                                 zbs[:, :])
                        else:
                            # tm band matches acc's partitions: walrus wants
                            # TT *inputs* on the same start partition
                            tm = scpool.tile([128, QT], f32, tag="tm", bufs=2)
                            nc.vector.tensor_mul(tm[hr, :], ots[hi][0:32, :],
                                                 zbs[:, :])
                            nc.gpsimd.tensor_add(ar, ar, tm[hr, :])

                def branch12(br, jqs):
                    kT = k96 if br == 1 else ks96
                    vt = v1 if br == 1 else vs
                    for jq in jqs:
                        nkb = 4 * (jq + 1) if br == 1 else 4
                        for hp in range(2):
                            ots = [
                                otp.tile([33, QT], f32, name=f"ot{i}",
                                         tag=f"ot{i}", bufs=1)
                                for i in range(2)
                            ]
                            for kb in range(nkb):
                                ksl = slice(kb * 128, kb * 128 + 128)
                                sg = sgp.tile([128, 1024], f32, tag="sg")
                                # on diagonal blocks only the causally-valid
                                # q columns [128i, QT) are ever consumed
                                off = (128 * (kb - 4 * jq)
                                       if br == 1 and kb >= 4 * jq else 0)
                                for hi in range(2):
                                    h = 2 * hp + hi
                                    nc.tensor.matmul(
                                        sg[:, hi * QT + off : hi * QT + QT],
                                        kT[:, h, ksl],
                                        q96[:, h,
                                            jq * QT + off : jq * QT + QT],
                                        start=True, stop=True,
                                    )
                                pt = ptpool.tile([128, 1024], bf16, tag="pt")
                                diag = br == 1 and kb >= 4 * jq
                                if diag:
                                    # exp only the causally-valid columns;
                                    # zero the rest, then mask the triangle
                                    i = kb - 4 * jq
                                    vq = slice(128 * i, QT)
                                    sgv = sg[:].rearrange(
                                        "p (h q) -> p h q", h=2)
                                    ptv = pt[:].rearrange(
                                        "p (h q) -> p h q", h=2)
                                    if i > 0:
                                        nc.gpsimd.memset(
                                            ptv[:, :, 0 : 128 * i], 0.0)
                                    nc.scalar.activation(
                                        ptv[:, :, vq], sgv[:, :, vq],
                                        EXP, scale=SCALE,
                                    )
                                    nc.gpsimd.affine_select(
                                        out=ptv[:, :, vq], in_=ptv[:, :, vq],
                                        compare_op=GE, fill=0.0,
                                        base=0,
                                        pattern=[[0, 2], [1, QT - 128 * i]],
                                        channel_multiplier=-1,
                                    )
                                else:
                                    nc.scalar.activation(
                                        pt[:], sg[:], EXP, scale=SCALE)
                                for hi in range(2):
                                    h = 2 * hp + hi
                                    nc.tensor.matmul(
                                        ots[hi][:],
                                        vt[:, kb, 33 * h : 33 * h + 33],
                                        pt[:, hi * QT : hi * QT + QT],
                                        start=(kb == 0), stop=(kb == nkb - 1),
                                    )
                            evict(jq, br, hp, ots)

                def branch3(jqs):
                    for jq in jqs:
                        for hp in range(2):
                            ots = [
                                otp.tile([33, QT], f32, name=f"ot{i}",
                                         tag=f"ot{i}", bufs=1)
                                for i in range(2)
                            ]
                            for qcp in range(2):  # pairs of 128-q chunks
                                sg = sgp.tile([128, 1024], f32, tag="sg")
                                for qcs in range(2):
                                    qb = 4 * jq + 2 * qcp + qcs
                                    qbs = slice(qb * 128, qb * 128 + 128)
                                    for hi in range(2):
                                        h = 2 * hp + hi
                                        for ki, kb in enumerate((qb - 1, qb)):
                                            col = slice(
                                                qcs * 512 + hi * 256 + ki * 128,
                                                qcs * 512 + hi * 256 + ki * 128
                                                + 128)
                                            if kb < 0:
                                                nc.vector.memset(
                                                    sg[:, col], 0.0)
                                                continue
                                            ksl = slice(kb * 128,
                                                        kb * 128 + 128)
                                            nc.tensor.matmul(
                                                sg[:, col], kw96[:, h, ksl],
                                                q96[:, h, qbs],
                                                start=True, stop=True,
                                            )
                                pt = ptpool.tile([128, 1024], bf16, tag="pt")
                                nc.scalar.activation(pt[:], sg[:], EXP,
                                                     scale=SCALE)
                                ptv = pt[:].rearrange("p (c q) -> p c q", c=8)
                                nc.gpsimd.affine_select(  # diag: col >= row
                                    out=ptv[:, slice(1, 8, 2)],
                                    in_=ptv[:, slice(1, 8, 2)],
                                    compare_op=GE, fill=0.0, base=0,
                                    pattern=[[0, 4], [1, 128]],
                                    channel_multiplier=-1,
                                )
                                nc.gpsimd.affine_select(  # prev: row > col
                                    out=ptv[:, slice(0, 8, 2)],
                                    in_=ptv[:, slice(0, 8, 2)],
                                    compare_op=GE, fill=0.0, base=-1,
                                    pattern=[[0, 4], [-1, 128]],
                                    channel_multiplier=1,
                                )
                                for qcs in range(2):
                                    qb = 4 * jq + 2 * qcp + qcs
                                    for hi in range(2):
                                        h = 2 * hp + hi
                                        for ki, kb in enumerate((qb - 1, qb)):
                                            if kb < 0:
                                                continue
                                            col = slice(
                                                qcs * 512 + hi * 256 + ki * 128,
                                                qcs * 512 + hi * 256 + ki * 128
                                                + 128)
                                            oc = slice(
                                                (2 * qcp + qcs) * 128,
                                                (2 * qcp + qcs) * 128 + 128)
                                            nc.tensor.matmul(
                                                ots[hi][:, oc],
                                                vw[:, kb,
                                                   33 * h : 33 * h + 33],
                                                pt[:, col],
                                                start=(kb == max(qb - 1, 0)),
                                                stop=(kb == qb),
                                            )
                            evict(jq, 3, hp, ots)

                # ---- emission: per-quad, q+b2 deps first (b2 is ACT-dense
                # and can start while the rest of the assembly runs on DVE).
                # Shared prep (cq/ckv/kr/tables) runs once; per-quad tiles
                # (q96, k96 nope rows, kw96, v*, acc) are rebuilt each quad.
                rmsnorm_proj(cqT, 96, WCQ, 96, ones96c, ones96r, 1.0 / 96)
                for hq in range(QUADS):
                    bo = hq * QW       # blob per-quad weight offset
                    b9 = hq * QW96     # b96 per-quad offset
                    b3 = hq * QW32     # b32 per-quad offset
                    proj_nope(q96,
                              lambda cc: s_b96[:, b9 + WQN : b9 + WQN + 128],
                              lambda cc, ts_: cqT[:, ts_], T, 1)
                    rope_all(q96,
                             lambda cc, hs: s_b96[:, asl(b9 + WQR, hs)],
                             lambda cc, hs: wqrS[hq][:, hs],
                             lambda cc, ts_: cqT[:, ts_], T, 1)
                    proj_nope(ks96,
                              lambda cc: s_blob[:, bo + WSKN + cc * 128 :
                                                bo + WSKN + cc * 128 + 128],
                              lambda cc, ts_: s_blob[:, asl(SEL + cc * KEEP, ts_)],
                              KEEP, 2)
                    rope_all(ks96,
                             lambda cc, hs: s_blob[:, asl(bo + WSKR + cc * 256, hs)],
                             lambda cc, hs: wskrS[hq][:, cc, hs],
                             lambda cc, ts_: s_blob[:, asl(SEL + cc * KEEP, ts_)],
                             KEEP, 2)
                    v_tile(
                        vs, 4,
                        lambda cc, tb: s_blob[:, SEL + cc * KEEP + tb * 128 :
                                              SEL + cc * KEEP + tb * 128 + 128],
                        lambda cc: s_blob[:, bo + WSV + cc * 128 :
                                          bo + WSV + cc * 128 + 128],
                        2,
                    )
                    branch12(2, range(NJQ))
                    if hq == 0:
                        # shared kv-path prep overlaps with branch 2
                        rmsnorm_proj(ckvT, 32, WCKV, 32, ones32c, ones32r,
                                     1.0 / 32)
                        kr_build()
                    proj_nope(k96, lambda cc: s_b32[:, b3 + WKN : b3 + WKN + 128],
                              lambda cc, ts_: ckvT[:, ts_], T, 1)
                    v_tile(
                        v1, NKB,
                        lambda cc, tb: ckvT[:, tb * 128 : tb * 128 + 128],
                        lambda cc: s_b32[:, b3 + WV : b3 + WV + 128], 1,
                    )
                    branch12(1, range(NJQ - 1))
                    proj_nope(kw96,
                              lambda cc: s_blob[:, bo + WWKN + cc * 128 :
                                                bo + WWKN + cc * 128 + 128],
                              lambda cc, ts_: xT_ap(cc, ts_), T, 2)
                    rope_all(kw96,
                             lambda cc, hs: s_blob[:, asl(bo + WWKR + cc * 256, hs)],
                             lambda cc, hs: wwkrS[hq][:, cc, hs],
                             lambda cc, ts_: xT_ap(cc, ts_), T, 2)
                    v_tile(
                        vw, NKB,
                        lambda cc, tb: xT_ap(cc, slice(tb * 128, tb * 128 + 128)),
                        lambda cc: s_blob[:, bo + WWV + cc * 128 :
                                          bo + WWV + cc * 128 + 128],
                        2,
                    )
                    branch3(range(NJQ))
                    branch12(1, [NJQ - 1])

                    # bf16 cast of this quad's o^T (split by jq)
                    for jq in range(NJQ):
                        oc = slice(jq * QT, jq * QT + QT)
                        nc.scalar.copy(oq[hq][:, oc], acc[:, oc])

                # ---- on-device Wproj: out[tok, C] = o^T.T @ Wproj ----
                for tb in range(NKB):
                    tsl = slice(tb * 128, tb * 128 + 128)
                    po = pp.tile([128, C], f32, tag="p1",
                                 padded_shape=[128, QT])
                    for k in range(QUADS):
                        nc.tensor.matmul(
                            po[:, :], oq[k][:, tsl],
                            s_blob[:, WP + k * C : WP + (k + 1) * C],
                            start=(k == 0), stop=(k == QUADS - 1),
                        )
                    ob = scpool.tile([128, C], bf16, tag="ob", bufs=2)
                    nc.vector.tensor_copy(ob[:, :], po[:, :])
                    _dma(d_out[tb], ob[:, :])

    if legalize:
        _legalize_pe_waits(nc, mybir)
    return nc


def _legalize_pe_waits(nc, mybir):
    """This walrus build encodes at most ONE sync-wait per compute
    instruction, but Tile emits up to 3. Split excess waits into standalone
    same-engine InstEventSemaphore waits placed immediately before the
    instruction (program point unchanged, so no deadlock risk)."""
    exempt = ("InstEventSemaphore", "InstNoOp",
              "InstUnconditionalBranch", "InstCall", "InstISA")
    for f in nc.m.functions:
        for bb in f.blocks:
            out = []
            changed = False
            for inst in bb.instructions:
                si = inst.sync_info
                tname = type(inst).__name__
                if si is not None and len(si.on_wait) > 1 and tname not in exempt:
                    for k, w in enumerate(si.on_wait[:-1]):
                        out.append(mybir.InstEventSemaphore(
                            name=f"{inst.name}-wsplit{k}",
                            engine=inst.engine,
                            ins=[], outs=[],
                            sync_info=mybir.SyncInfo(
                                on_wait=[w], on_update=[]),
                        ))
                    inst.sync_info = mybir.SyncInfo(
                        on_wait=[si.on_wait[-1]],
                        on_update=list(si.on_update),
                    )
                    changed = True
                out.append(inst)
            if changed:
                bb.instructions = out


def _get_nc():
    if "nc" not in _CACHE:
        nc = _build_bass()
        # bass2jax re-serializes the (frozen) BIR on every jit lowering;
        # memoize the bytes on this instance to keep per-call lower cheap
        try:
            raw = nc.to_json_bytes()
            nc.to_json_bytes = lambda: raw
        except Exception:
            pass
        _CACHE["nc"] = nc
    return _CACHE["nc"]


# ---------------------------------------------------------------------------
# host orchestration
# ---------------------------------------------------------------------------

def _prep_in_maps(inputs):
    x = np.asarray(inputs["x"], np.float32)
    cos, sin = _freqs(T)  # [T, 32] each
    c32 = np.ascontiguousarray(cos.T)  # [32, T]
    s32 = np.ascontiguousarray(sin.T)

    qnw = np.asarray(inputs["q_norm_w"], np.float32)
    kvw = np.asarray(inputs["kv_norm_w"], np.float32)
    Wqn_r = (np.asarray(inputs["Wq_nope"], np.float32)
             * qnw[:, None]).reshape(96, N_HEAD, NOPE)
    Wqr_r = (np.asarray(inputs["Wq_rope"], np.float32)
             * qnw[:, None]).reshape(96, N_HEAD, ROPE)
    Wkn_r = (np.asarray(inputs["Wk_nope"], np.float32)
             * kvw[:, None]).reshape(32, N_HEAD, NOPE)
    Wv_r = (np.asarray(inputs["Wv"], np.float32)
            * kvw[:, None]).reshape(32, N_HEAD, VDIM)
    Wsk_r = np.asarray(inputs["Wsel_k"], np.float32).reshape(C, N_HEAD, HD)
    Wsv_r = np.asarray(inputs["Wsel_v"], np.float32).reshape(C, N_HEAD, VDIM)
    Wwk_r = np.asarray(inputs["Wwin_k"], np.float32).reshape(C, N_HEAD, HD)
    Wwv_r = np.asarray(inputs["Wwin_v"], np.float32).reshape(C, N_HEAD, VDIM)
    Wkr = np.asarray(inputs["Wk_rope"], np.float32) / N_HEAD  # [C, 64]
    Wcq = np.asarray(inputs["Wcq"], np.float32)
    Wckv = np.asarray(inputs["Wckv"], np.float32)
    Wp = np.asarray(inputs["Wproj"], np.float32)

    per_batch = []
    for b in range(B):
        xb = x[b]
        glog = (xb @ np.asarray(inputs["Wgate"], np.float32)).mean(0)
        g = np.exp(glog - glog.max())
        g = (g / g.sum()).astype(np.float32)
        scores = (xb @ np.asarray(inputs["W_imp"], np.float32))[:, 0]
        idx = np.sort(np.argpartition(-scores, KEEP - 1)[:KEEP])
        per_batch.append((xb, xb[idx], g))

    in_maps = []
    for core in range(N_CORES):
        b, hg2 = divmod(core, N_CORES // B)
        xb, sel, g = per_batch[b]

        blob = np.zeros((128, NB), np.float32)

        def put(col, w):  # w [C, X] -> 2 cin chunks side by side
            Xw = w.shape[1]
            for cc in range(2):
                blob[:, col + cc * Xw : col + (cc + 1) * Xw] = \
                    w[cc * 128 : (cc + 1) * 128]

        put(XT, xb.T)
        put(SEL, sel.T)
        b96_parts, b32_parts = [], [c32, s32]
        for hq in range(QUADS):
            gq = hg2 * QUADS + hq  # global head-quad
            hsl = slice(gq * HPC, gq * HPC + HPC)
            bo = hq * QW
            put(bo + WSKN, Wsk_r[:, hsl, :NOPE].reshape(C, -1))
            put(bo + WSKR, Wsk_r[:, hsl, NOPE:].reshape(C, -1))
            put(bo + WSV, (Wsv_r[:, hsl] * g[1]).reshape(C, -1))
            put(bo + WWKN, Wwk_r[:, hsl, :NOPE].reshape(C, -1))
            put(bo + WWKR, Wwk_r[:, hsl, NOPE:].reshape(C, -1))
            put(bo + WWV, (Wwv_r[:, hsl] * g[2]).reshape(C, -1))
            b96_parts += [Wqn_r[:, hsl].reshape(96, -1),
                          Wqr_r[:, hsl].reshape(96, -1)]
            b32_parts += [Wkn_r[:, hsl].reshape(32, -1),
                          (Wv_r[:, hsl] * g[0]).reshape(32, -1)]
        put(WCQ, Wcq)
        put(WCKV, Wckv)
        put(WKR, Wkr)
        for k in range(4):  # Wproj [512, C] -> 4 contraction chunks
            blob[:, WP + k * C : WP + (k + 1) * C] = \
                Wp[k * 128 : (k + 1) * 128]

        b96 = np.concatenate(b96_parts, 1)
        b32 = np.concatenate(b32_parts, 1)

        in_maps.append({
            "blob": blob.astype(BF),
            "b96": np.ascontiguousarray(b96).astype(BF),
            "b32": np.ascontiguousarray(b32).astype(BF),
        })
    return in_maps, Wp


def _run(inputs, trace=False):
    from concourse.bass_utils import run_bass_kernel_spmd

    nc = _get_nc()
    in_maps, Wp = _prep_in_maps(inputs)
    res = run_bass_kernel_spmd(nc, in_maps, list(range(N_CORES)), trace=trace)
    out = np.zeros((B, T, C), np.float32)
    for core in range(N_CORES):
        out[core] = np.asarray(
            res.results[core]["outT"], np.float32).reshape(T, C)
    return out, res


def kernel(**inputs):
    out, _ = _run(inputs, trace=False)
    return out
